# revision 1
# baseline (speedup 1.0000x reference)
"""DenseShift forward kernel for Trainium2 (8 NeuronCores, data-parallel).

Computes y = x @ W + bias where W = 2^shift * (-1)^sign, for
x: [524288, 256] f32, shift/sign: [256, 256], bias: [1, 256].

Sharding: x is split along batch across 8 cores (65536 rows each);
shift/sign/bias are replicated. No collectives (forward only).

Per-core dataflow (memory-bound problem; the point is streaming x/y at
HBM rate while the PE keeps up):
  - W is reconstructed exactly on-device with integer bit ops:
    bits = ((shift + 127) << 23) | (sign << 31), bitcast to f32.
  - x is DMA'd in 2 MiB groups (16 b-tiles of [128, 256]).
  - Each [128, 128] chunk of x is transposed on the PE (is_transpose
    passthrough, exact) into PSUM, then moved to SBUF by the DVE.
  - Matmul precision tiers:
      "tf32":   xT rounded to float32r (TF32), 2 matmuls per b-tile.
      "tf32x2": hi/lo TF32 split of xT (Kahan residual), 4 matmuls —
                ~2^-22 relative accuracy at half the cost of fp32.
      "fp32":   plain fp32 matmuls (4 cycles/row on the PE).
    W entries are powers of two, exact in every tier.
  - bias add is fused into the mandatory PSUM->SBUF DVE copy of y.
  - y written back in 2 MiB groups.
"""

import numpy as np

import concourse.mybir as mybir
import concourse.tile as tile
from concourse import bacc
from concourse.bass_utils import run_bass_kernel_spmd
from concourse.masks import make_identity

N_CORES = 8
BATCH, IN_DIM, OUT_DIM = 524288, 256, 256
B_CORE = BATCH // N_CORES  # 65536 rows per core
PRECISION = "tf32x2"

F32 = mybir.dt.float32
F32R = mybir.dt.float32r
I32 = mybir.dt.int32


def build_bass(
    b_core: int = B_CORE,
    group_tiles: int = 16,
    precision: str = PRECISION,
    repeats: int = 1,
    act_hi_copy: bool = False,
    bufs_in: int = 3,
    bufs_out: int = 3,
    bufs_xt: int = 3,
    bufs_pst: int = 2,
    bufs_psy: int = 2,
    nb: int = 4,
    hi_slices: int = 2,
    lo_bf16: bool = False,
    t_f32r: bool = False,
    hi_fp16: bool = False,
    out_dma_scalar: bool = False,
    dma_split: int = 1,
) -> "bacc.Bacc":
    """Build the per-core SPMD Bass program.

    repeats > 1 re-runs the whole main loop (identical writes) — used only
    for differential timing in the dev harness.
    act_hi_copy moves the PSUM->SBUF hi-cast from the DVE to the scalar
    (ACT) engine to relieve DVE pressure.
    nb = b-tiles batched per DVE/ACT op (PSUM tiles span nb*1KB/partition;
    nb=4 -> 2 banks per PSUM tile). Amortizes the ~230 ns PSUM access
    latency each DVE op pays.
    """
    P = 128
    G = group_tiles
    assert G % nb == 0
    assert b_core % (P * G) == 0
    n_groups = b_core // (P * G)
    mm_dt = F32 if precision == "fp32" else F32R
    if hi_fp16:
        assert precision == "tf32x2" and lo_bf16
        mm_dt = mybir.dt.float16

    nc = bacc.Bacc(
        "TRN2", target_bir_lowering=False, debug=False, num_devices=N_CORES
    )
    x = nc.dram_tensor("x", [b_core, IN_DIM], F32, kind="ExternalInput").ap()
    shift = nc.dram_tensor("shift", [IN_DIM, OUT_DIM], F32, kind="ExternalInput").ap()
    sign = nc.dram_tensor("sign", [IN_DIM, OUT_DIM], F32, kind="ExternalInput").ap()
    bias = nc.dram_tensor("bias", [1, OUT_DIM], F32, kind="ExternalInput").ap()
    y = nc.dram_tensor("y", [b_core, OUT_DIM], F32, kind="ExternalOutput").ap()

    # [g, p, t, m] views: group g covers rows [g*G*128, (g+1)*G*128).
    # Partition p holds rows {g*G*128 + p*G + t}: per-partition DRAM chunks
    # are G*1KB contiguous (16 KB at G=16), so DMA descriptors hit full
    # packet size. The row permutation is identical on input and output, so
    # it cancels (each b-tile is just a permuted set of 128 rows).
    x_v = x.rearrange("(g p t) m -> g p t m", p=P, t=G)
    y_v = y.rearrange("(g p t) m -> g p t m", p=P, t=G)

    with tile.TileContext(nc) as tc:
        with (
            tc.tile_pool(name="const", bufs=1) as const_pool,
            tc.tile_pool(name="xin", bufs=bufs_in) as in_pool,
            tc.tile_pool(name="yout", bufs=bufs_out) as out_pool,
            tc.tile_pool(name="xt", bufs=bufs_xt) as xt_pool,
            tc.tile_pool(name="pst", bufs=bufs_pst, space="PSUM") as psum_t_pool,
            tc.tile_pool(name="psy", bufs=bufs_psy, space="PSUM") as psum_y_pool,
        ):
            # ---- constants ----
            ident = const_pool.tile([P, P], F32R if t_f32r else F32)
            make_identity(nc, ident[:])

            # W = 2^shift * (-1)^sign, exactly, via exponent-field bits.
            # Layout: w[:, c*256:(c+1)*256] = W[c*128:(c+1)*128, :]
            sh = const_pool.tile([P, 2 * OUT_DIM], F32)
            sg = const_pool.tile([P, 2 * OUT_DIM], F32)
            for c in range(2):
                cs = slice(c * OUT_DIM, (c + 1) * OUT_DIM)
                rs = slice(c * P, (c + 1) * P)
                nc.sync.dma_start(sh[:, cs], shift[rs, :])
                nc.sync.dma_start(sg[:, cs], sign[rs, :])
            sh_i = const_pool.tile([P, 2 * OUT_DIM], I32)
            sg_i = const_pool.tile([P, 2 * OUT_DIM], I32)
            w_i = const_pool.tile([P, 2 * OUT_DIM], I32)
            # biased exponent (shift + 127), still f32 -> int32 (exact ints)
            nc.vector.tensor_scalar_add(sh[:], sh[:], 127.0)
            nc.vector.tensor_copy(sh_i[:], sh[:])
            nc.vector.tensor_copy(sg_i[:], sg[:])
            nc.vector.tensor_scalar(
                sh_i[:], sh_i[:], 23, None, op0=mybir.AluOpType.logical_shift_left
            )
            nc.vector.tensor_scalar(
                sg_i[:], sg_i[:], 31, None, op0=mybir.AluOpType.logical_shift_left
            )
            nc.vector.tensor_tensor(
                w_i[:], sh_i[:], sg_i[:], op=mybir.AluOpType.bitwise_or
            )
            # materialize W at the matmul dtype (values are powers of two,
            # exact under TF32 rounding)
            w_mm = const_pool.tile([P, 2 * OUT_DIM], mm_dt)
            nc.vector.tensor_copy(w_mm[:], w_i[:].bitcast(F32))
            w_lo = w_mm
            lo_dt = F32R
            if lo_bf16:
                lo_dt = mybir.dt.bfloat16
                w_lo = const_pool.tile([P, 2 * OUT_DIM], lo_dt)
                nc.vector.tensor_copy(w_lo[:], w_i[:].bitcast(F32))

            # bias broadcast to all 128 partitions via a K=1 matmul of
            # ones[1,128].T @ bias[1,256], then tiled nb times along free
            ones = const_pool.tile([1, P], F32)
            nc.gpsimd.memset(ones[:], 1.0)
            bias_row = const_pool.tile([1, OUT_DIM], F32)
            nc.sync.dma_start(bias_row[:], bias[:])
            bias_bc = const_pool.tile([P, nb, OUT_DIM], F32)
            psum_b = psum_t_pool.tile([P, OUT_DIM], F32, tag="ps_t")
            nc.tensor.matmul(psum_b[:], ones[:], bias_row[:], start=True, stop=True)
            for q in range(nb):
                nc.vector.tensor_copy(bias_bc[:, q, :], psum_b[:])

            # ---- main loop ----
            for g in range(n_groups * repeats):
                g = g % n_groups
                x_in = in_pool.tile([P, G, IN_DIM], F32)
                dsz = G // dma_split
                for s in range(dma_split):
                    nc.sync.dma_start(
                        x_in[:, s * dsz : (s + 1) * dsz, :],
                        x_v[g][:, s * dsz : (s + 1) * dsz, :],
                    )
                y_out = out_pool.tile([P, G, OUT_DIM], F32)
                for t0 in range(0, G, nb):
                    # transpose 2*nb x chunks into one batched PSUM tile
                    ps_t = psum_t_pool.tile([P, nb, IN_DIM], F32, tag="ps_t")
                    for q in range(nb):
                        for c in range(2):
                            t_out = ps_t[:, q, c * P : (c + 1) * P]
                            t_in = x_in[:, t0 + q, c * P : (c + 1) * P]
                            if t_f32r:
                                t_out = t_out.bitcast(F32R)
                                t_in = t_in.bitcast(F32R)
                            nc.tensor.transpose(t_out, t_in, ident[:])
                    # hi-cast and (for tf32x2) residual computed in
                    # half-block slices so the lo matmuls unblock earlier
                    xT = xt_pool.tile([P, nb, IN_DIM], mm_dt, tag="xt_hi")
                    xT_lo = None
                    if precision == "tf32x2":
                        xT_lo = xt_pool.tile([P, nb, IN_DIM], lo_dt, tag="xt_lo")
                    h_step = max(nb // hi_slices, 1)
                    for h0 in range(0, nb, h_step):
                        hs = slice(h0, h0 + h_step)
                        if act_hi_copy:
                            nc.scalar.activation(
                                xT[:, hs, :],
                                ps_t[:, hs, :],
                                mybir.ActivationFunctionType.Copy,
                            )
                        else:
                            nc.vector.tensor_copy(xT[:, hs, :], ps_t[:, hs, :])
                        if xT_lo is not None:
                            nc.vector.tensor_tensor(
                                xT_lo[:, hs, :],
                                ps_t[:, hs, :],
                                xT[:, hs, :],
                                op=mybir.AluOpType.subtract,
                            )
                    ps_y = psum_y_pool.tile([P, nb, OUT_DIM], F32)
                    # per-q accumulation groups must stay contiguous:
                    # start=True resets the whole PSUM zero-region, so
                    # interleaving open groups in one bank corrupts results
                    for q in range(nb):
                        parts = [(xT, 0, w_mm), (xT, 1, w_mm)]
                        if xT_lo is not None:
                            parts += [(xT_lo, 0, w_lo), (xT_lo, 1, w_lo)]
                        for i, (src, c, w_use) in enumerate(parts):
                            nc.tensor.matmul(
                                ps_y[:, q, :],
                                src[:, q, c * P : (c + 1) * P],
                                w_use[:, c * OUT_DIM : (c + 1) * OUT_DIM],
                                start=(i == 0),
                                stop=(i == len(parts) - 1),
                            )
                    # fused bias-add + PSUM->SBUF move, batched over nb tiles
                    nc.vector.tensor_add(
                        y_out[:, t0 : t0 + nb, :], ps_y[:], bias_bc[:]
                    )
                out_eng = nc.scalar if out_dma_scalar else nc.sync
                for s in range(dma_split):
                    out_eng.dma_start(
                        y_v[g][:, s * dsz : (s + 1) * dsz, :],
                        y_out[:, s * dsz : (s + 1) * dsz, :],
                    )
    nc.compile()
    return nc


_NC_CACHE: dict = {}


def _get_nc():
    if "nc" not in _NC_CACHE:
        _NC_CACHE["nc"] = build_bass()
    return _NC_CACHE["nc"]


def kernel(x, shift, sign, bias):
    x = np.ascontiguousarray(x, dtype=np.float32)
    shift = np.ascontiguousarray(shift, dtype=np.float32)
    sign = np.ascontiguousarray(sign, dtype=np.float32)
    bias = np.ascontiguousarray(bias, dtype=np.float32)
    assert x.shape == (BATCH, IN_DIM)

    nc = _get_nc()
    shards = np.split(x, N_CORES, axis=0)
    in_maps = [
        {"x": shards[c], "shift": shift, "sign": sign, "bias": bias}
        for c in range(N_CORES)
    ]
    res = run_bass_kernel_spmd(nc, in_maps, core_ids=list(range(N_CORES)))
    return np.concatenate([r["y"] for r in res.results], axis=0)



# revision 19
# speedup vs baseline: 18.9940x; 18.9940x over previous
"""DenseShift forward kernel for Trainium2 (8 NeuronCores, data-parallel).

Computes y = x @ W + bias where W = 2^shift * (-1)^sign, for
x: [524288, 256] f32, shift/sign: [256, 256], bias: [1, 256].

Sharding: x is split along batch across 8 cores (65536 rows each);
shift/sign/bias are replicated. No collectives (forward only).

Active kernel: build_bass_v2 (fp16 pipeline, ~305 us/core HW time vs
~500 us for the tf32x2 v1 below, vs a ~375 us f32-io DMA roofline).
Per-core dataflow, per 2 MiB group of 16 b-tiles [128 rows, 256]:
  - W is reconstructed exactly on-device with integer bit ops:
    bits = ((shift + 127) << 23) | (sign << 31), bitcast f32, then
    materialized as fp16 (powers of two 2^-10..2^-1 are exact in fp16).
  - ACT casts x f32 -> fp16 (the only lossy step: x rounds at 2^-11;
    the fp16 matmul then accumulates exactly in f32 PSUM, so end-to-end
    max rel err is ~4e-4 against the 2e-2 gate).
  - PE transposes each [128,128] fp16 chunk (1 cycle/row vs 2 for f32)
    into PSUM; DVE moves it to SBUF (2-byte packed -> half-rate).
  - 2 fp16 matmuls per b-tile (K-chunks of 128), f32 PSUM accumulate.
  - DVE writes y = PSUM + bias to SBUF as fp16, halving write traffic
    (y rounds at 2^-11, well inside the gate); host upcasts to f32.
  - Input DMA issues on the sync queue, output on the gpsimd queue, so
    the two streams never head-of-line block (16 HW DMA engines serve
    both; observed ~430 GB/s/core aggregate peak).
  - The (group, t-block) loop is software-pipelined: PE issue order is
    [transposes(t), matmuls(t-1)] so the PE never waits on the DVE
    PSUM->SBUF move of the block it just transposed.

v1 (build_bass, kept for reference/A-B): exact-ish tf32x2 path, f32 y.
"""

import numpy as np

import concourse.mybir as mybir
import concourse.tile as tile
from concourse import bacc
from concourse.bass_utils import run_bass_kernel_spmd
from concourse.masks import make_identity

N_CORES = 8
BATCH, IN_DIM, OUT_DIM = 524288, 256, 256
B_CORE = BATCH // N_CORES  # 65536 rows per core
PRECISION = "tf32x2"

F32 = mybir.dt.float32
F32R = mybir.dt.float32r
I32 = mybir.dt.int32


def build_bass(
    b_core: int = B_CORE,
    group_tiles: int = 16,
    precision: str = PRECISION,
    repeats: int = 1,
    act_hi_copy: bool = False,
    bufs_in: int = 3,
    bufs_out: int = 3,
    bufs_xt: int = 3,
    bufs_pst: int = 2,
    bufs_psy: int = 2,
    nb: int = 4,
    hi_slices: int = 2,
    lo_bf16: bool = False,
    t_f32r: bool = False,
    hi_fp16: bool = False,
    out_dma_scalar: bool = False,
    dma_split: int = 1,
) -> "bacc.Bacc":
    """Build the per-core SPMD Bass program.

    repeats > 1 re-runs the whole main loop (identical writes) — used only
    for differential timing in the dev harness.
    act_hi_copy moves the PSUM->SBUF hi-cast from the DVE to the scalar
    (ACT) engine to relieve DVE pressure.
    nb = b-tiles batched per DVE/ACT op (PSUM tiles span nb*1KB/partition;
    nb=4 -> 2 banks per PSUM tile). Amortizes the ~230 ns PSUM access
    latency each DVE op pays.
    """
    P = 128
    G = group_tiles
    assert G % nb == 0
    assert b_core % (P * G) == 0
    n_groups = b_core // (P * G)
    mm_dt = F32 if precision == "fp32" else F32R
    if hi_fp16:
        assert precision == "tf32x2" and lo_bf16
        mm_dt = mybir.dt.float16

    nc = bacc.Bacc(
        "TRN2", target_bir_lowering=False, debug=False, num_devices=N_CORES
    )
    x = nc.dram_tensor("x", [b_core, IN_DIM], F32, kind="ExternalInput").ap()
    shift = nc.dram_tensor("shift", [IN_DIM, OUT_DIM], F32, kind="ExternalInput").ap()
    sign = nc.dram_tensor("sign", [IN_DIM, OUT_DIM], F32, kind="ExternalInput").ap()
    bias = nc.dram_tensor("bias", [1, OUT_DIM], F32, kind="ExternalInput").ap()
    y = nc.dram_tensor("y", [b_core, OUT_DIM], F32, kind="ExternalOutput").ap()

    # [g, p, t, m] views: group g covers rows [g*G*128, (g+1)*G*128).
    # Partition p holds rows {g*G*128 + p*G + t}: per-partition DRAM chunks
    # are G*1KB contiguous (16 KB at G=16), so DMA descriptors hit full
    # packet size. The row permutation is identical on input and output, so
    # it cancels (each b-tile is just a permuted set of 128 rows).
    x_v = x.rearrange("(g p t) m -> g p t m", p=P, t=G)
    y_v = y.rearrange("(g p t) m -> g p t m", p=P, t=G)

    with tile.TileContext(nc) as tc:
        with (
            tc.tile_pool(name="const", bufs=1) as const_pool,
            tc.tile_pool(name="xin", bufs=bufs_in) as in_pool,
            tc.tile_pool(name="yout", bufs=bufs_out) as out_pool,
            tc.tile_pool(name="xt", bufs=bufs_xt) as xt_pool,
            tc.tile_pool(name="pst", bufs=bufs_pst, space="PSUM") as psum_t_pool,
            tc.tile_pool(name="psy", bufs=bufs_psy, space="PSUM") as psum_y_pool,
        ):
            # ---- constants ----
            ident = const_pool.tile([P, P], F32R if t_f32r else F32)
            make_identity(nc, ident[:])

            # W = 2^shift * (-1)^sign, exactly, via exponent-field bits.
            # Layout: w[:, c*256:(c+1)*256] = W[c*128:(c+1)*128, :]
            sh = const_pool.tile([P, 2 * OUT_DIM], F32)
            sg = const_pool.tile([P, 2 * OUT_DIM], F32)
            for c in range(2):
                cs = slice(c * OUT_DIM, (c + 1) * OUT_DIM)
                rs = slice(c * P, (c + 1) * P)
                nc.sync.dma_start(sh[:, cs], shift[rs, :])
                nc.sync.dma_start(sg[:, cs], sign[rs, :])
            sh_i = const_pool.tile([P, 2 * OUT_DIM], I32)
            sg_i = const_pool.tile([P, 2 * OUT_DIM], I32)
            w_i = const_pool.tile([P, 2 * OUT_DIM], I32)
            # biased exponent (shift + 127), still f32 -> int32 (exact ints)
            nc.vector.tensor_scalar_add(sh[:], sh[:], 127.0)
            nc.vector.tensor_copy(sh_i[:], sh[:])
            nc.vector.tensor_copy(sg_i[:], sg[:])
            nc.vector.tensor_scalar(
                sh_i[:], sh_i[:], 23, None, op0=mybir.AluOpType.logical_shift_left
            )
            nc.vector.tensor_scalar(
                sg_i[:], sg_i[:], 31, None, op0=mybir.AluOpType.logical_shift_left
            )
            nc.vector.tensor_tensor(
                w_i[:], sh_i[:], sg_i[:], op=mybir.AluOpType.bitwise_or
            )
            # materialize W at the matmul dtype (values are powers of two,
            # exact under TF32 rounding)
            w_mm = const_pool.tile([P, 2 * OUT_DIM], mm_dt)
            nc.vector.tensor_copy(w_mm[:], w_i[:].bitcast(F32))
            w_lo = w_mm
            lo_dt = F32R
            if lo_bf16:
                lo_dt = mybir.dt.bfloat16
                w_lo = const_pool.tile([P, 2 * OUT_DIM], lo_dt)
                nc.vector.tensor_copy(w_lo[:], w_i[:].bitcast(F32))

            # bias broadcast to all 128 partitions via a K=1 matmul of
            # ones[1,128].T @ bias[1,256], then tiled nb times along free
            ones = const_pool.tile([1, P], F32)
            nc.gpsimd.memset(ones[:], 1.0)
            bias_row = const_pool.tile([1, OUT_DIM], F32)
            nc.sync.dma_start(bias_row[:], bias[:])
            bias_bc = const_pool.tile([P, nb, OUT_DIM], F32)
            psum_b = psum_t_pool.tile([P, OUT_DIM], F32, tag="ps_t")
            nc.tensor.matmul(psum_b[:], ones[:], bias_row[:], start=True, stop=True)
            for q in range(nb):
                nc.vector.tensor_copy(bias_bc[:, q, :], psum_b[:])

            # ---- main loop ----
            for g in range(n_groups * repeats):
                g = g % n_groups
                x_in = in_pool.tile([P, G, IN_DIM], F32)
                dsz = G // dma_split
                for s in range(dma_split):
                    nc.sync.dma_start(
                        x_in[:, s * dsz : (s + 1) * dsz, :],
                        x_v[g][:, s * dsz : (s + 1) * dsz, :],
                    )
                y_out = out_pool.tile([P, G, OUT_DIM], F32)
                for t0 in range(0, G, nb):
                    # transpose 2*nb x chunks into one batched PSUM tile
                    ps_t = psum_t_pool.tile([P, nb, IN_DIM], F32, tag="ps_t")
                    for q in range(nb):
                        for c in range(2):
                            t_out = ps_t[:, q, c * P : (c + 1) * P]
                            t_in = x_in[:, t0 + q, c * P : (c + 1) * P]
                            if t_f32r:
                                t_out = t_out.bitcast(F32R)
                                t_in = t_in.bitcast(F32R)
                            nc.tensor.transpose(t_out, t_in, ident[:])
                    # hi-cast and (for tf32x2) residual computed in
                    # half-block slices so the lo matmuls unblock earlier
                    xT = xt_pool.tile([P, nb, IN_DIM], mm_dt, tag="xt_hi")
                    xT_lo = None
                    if precision == "tf32x2":
                        xT_lo = xt_pool.tile([P, nb, IN_DIM], lo_dt, tag="xt_lo")
                    h_step = max(nb // hi_slices, 1)
                    for h0 in range(0, nb, h_step):
                        hs = slice(h0, h0 + h_step)
                        if act_hi_copy:
                            nc.scalar.activation(
                                xT[:, hs, :],
                                ps_t[:, hs, :],
                                mybir.ActivationFunctionType.Copy,
                            )
                        else:
                            nc.vector.tensor_copy(xT[:, hs, :], ps_t[:, hs, :])
                        if xT_lo is not None:
                            nc.vector.tensor_tensor(
                                xT_lo[:, hs, :],
                                ps_t[:, hs, :],
                                xT[:, hs, :],
                                op=mybir.AluOpType.subtract,
                            )
                    ps_y = psum_y_pool.tile([P, nb, OUT_DIM], F32)
                    # per-q accumulation groups must stay contiguous:
                    # start=True resets the whole PSUM zero-region, so
                    # interleaving open groups in one bank corrupts results
                    for q in range(nb):
                        parts = [(xT, 0, w_mm), (xT, 1, w_mm)]
                        if xT_lo is not None:
                            parts += [(xT_lo, 0, w_lo), (xT_lo, 1, w_lo)]
                        for i, (src, c, w_use) in enumerate(parts):
                            nc.tensor.matmul(
                                ps_y[:, q, :],
                                src[:, q, c * P : (c + 1) * P],
                                w_use[:, c * OUT_DIM : (c + 1) * OUT_DIM],
                                start=(i == 0),
                                stop=(i == len(parts) - 1),
                            )
                    # fused bias-add + PSUM->SBUF move, batched over nb tiles
                    nc.vector.tensor_add(
                        y_out[:, t0 : t0 + nb, :], ps_y[:], bias_bc[:]
                    )
                out_eng = nc.scalar if out_dma_scalar else nc.sync
                for s in range(dma_split):
                    out_eng.dma_start(
                        y_v[g][:, s * dsz : (s + 1) * dsz, :],
                        y_out[:, s * dsz : (s + 1) * dsz, :],
                    )
    nc.compile()
    return nc


def build_bass_v2(
    b_core: int = B_CORE,
    group_tiles: int = 16,
    nb: int = 4,
    repeats: int = 1,
    bufs_in: int = 3,
    bufs_out: int = 3,
    bufs_xh: int = 2,
    bufs_xt: int = 3,
    bufs_pst: int = 2,
    bufs_psy: int = 3,
    cast_eng: str = "scalar",
    move_eng: str = "vector",
    in_dma: str = "sync",
    out_dma: str = "gpsimd",
    y_fp16: bool = False,
    mm_dt_name: str = "float16",
    y_gp: int = 0,
    move_act: int = 0,
    pipe: int = 1,
    cast_split: int = 1,
    cast_lead: int = 2,
    out_split: int = 1,
) -> "bacc.Bacc":
    """fp16 single-matmul pipeline (v2).

    Per b-tile of 128 rows: x cast f32->fp16 on ACT (group-batched), two
    fp16 PE transposes (1 cycle/row vs 2 for f32), DVE PSUM->SBUF move of
    xT (2-byte packed -> half-rate cycles), two fp16 matmuls (W is powers
    of two -> exact in fp16; only x rounds, ~2^-11), DVE y = PSUM + bias.

    The t-block loop is software-pipelined by one stage: PE issue order is
    [transposes(t), matmuls(t-1)] so the PE never waits on the DVE move of
    the block it just transposed.
    """
    P = 128
    G = group_tiles
    assert G % nb == 0
    assert b_core % (P * G) == 0
    n_groups = b_core // (P * G)
    nt = G // nb
    mm_dt = getattr(mybir.dt, mm_dt_name)
    y_dt = mybir.dt.float16 if y_fp16 else F32

    nc = bacc.Bacc(
        "TRN2", target_bir_lowering=False, debug=False, num_devices=N_CORES
    )
    x = nc.dram_tensor("x", [b_core, IN_DIM], F32, kind="ExternalInput").ap()
    shift = nc.dram_tensor("shift", [IN_DIM, OUT_DIM], F32, kind="ExternalInput").ap()
    sign = nc.dram_tensor("sign", [IN_DIM, OUT_DIM], F32, kind="ExternalInput").ap()
    bias = nc.dram_tensor("bias", [1, OUT_DIM], F32, kind="ExternalInput").ap()
    y = nc.dram_tensor("y", [b_core, OUT_DIM], y_dt, kind="ExternalOutput").ap()

    x_v = x.rearrange("(g p t) m -> g p t m", p=P, t=G)
    y_v = y.rearrange("(g p t) m -> g p t m", p=P, t=G)

    def eng(name):
        return getattr(nc, name)

    with tile.TileContext(nc) as tc:
        with (
            tc.tile_pool(name="const", bufs=1) as const_pool,
            tc.tile_pool(name="xin", bufs=bufs_in) as in_pool,
            tc.tile_pool(name="xh", bufs=bufs_xh) as xh_pool,
            tc.tile_pool(name="yout", bufs=bufs_out) as out_pool,
            tc.tile_pool(name="xt", bufs=bufs_xt) as xt_pool,
            tc.tile_pool(name="pst", bufs=bufs_pst, space="PSUM") as psum_t_pool,
            tc.tile_pool(name="psy", bufs=bufs_psy, space="PSUM") as psum_y_pool,
        ):
            # ---- constants ----
            ident = const_pool.tile([P, P], mm_dt)
            make_identity(nc, ident[:])

            # W = 2^shift * (-1)^sign via exponent-field bits (exact), then
            # materialized at fp16 (powers of two are exact in fp16).
            sh = const_pool.tile([P, 2 * OUT_DIM], F32)
            sg = const_pool.tile([P, 2 * OUT_DIM], F32)
            for c in range(2):
                cs = slice(c * OUT_DIM, (c + 1) * OUT_DIM)
                rs = slice(c * P, (c + 1) * P)
                nc.sync.dma_start(sh[:, cs], shift[rs, :])
                nc.sync.dma_start(sg[:, cs], sign[rs, :])
            sh_i = const_pool.tile([P, 2 * OUT_DIM], I32)
            sg_i = const_pool.tile([P, 2 * OUT_DIM], I32)
            w_i = const_pool.tile([P, 2 * OUT_DIM], I32)
            nc.vector.tensor_scalar_add(sh[:], sh[:], 127.0)
            nc.vector.tensor_copy(sh_i[:], sh[:])
            nc.vector.tensor_copy(sg_i[:], sg[:])
            nc.vector.tensor_scalar(
                sh_i[:], sh_i[:], 23, None, op0=mybir.AluOpType.logical_shift_left
            )
            nc.vector.tensor_scalar(
                sg_i[:], sg_i[:], 31, None, op0=mybir.AluOpType.logical_shift_left
            )
            nc.vector.tensor_tensor(
                w_i[:], sh_i[:], sg_i[:], op=mybir.AluOpType.bitwise_or
            )
            w_mm = const_pool.tile([P, 2 * OUT_DIM], mm_dt)
            nc.vector.tensor_copy(w_mm[:], w_i[:].bitcast(F32))

            # bias broadcast [P, nb, OUT_DIM] via K=1 matmul of ones.T @ bias
            ones = const_pool.tile([1, P], F32)
            nc.gpsimd.memset(ones[:], 1.0)
            bias_row = const_pool.tile([1, OUT_DIM], F32)
            nc.sync.dma_start(bias_row[:], bias[:])
            bias_bc = const_pool.tile([P, nb, OUT_DIM], F32)
            psum_b = psum_y_pool.tile([P, nb, OUT_DIM], F32, tag="ps_y")
            nc.tensor.matmul(psum_b[:, 0, :], ones[:], bias_row[:], start=True, stop=True)
            for q in range(nb):
                nc.vector.tensor_copy(bias_bc[:, q, :], psum_b[:, 0, :])

            # ---- main loop: flat over (group, t-block), pipelined by `pipe`
            # stages (matmuls trail transposes so the PE never waits on the
            # DVE move of the block it just transposed) ----
            xh_tiles: dict = {}
            y_tiles: dict = {}
            pending: list = []  # (g, t, xT_tile, ps_t_tile)

            def issue_matmuls(pv):
                g0, t0, xT0, _ = pv
                ps_y = psum_y_pool.tile([P, nb, OUT_DIM], F32, tag="ps_y")
                for q in range(nb):
                    for i, c in enumerate((0, 1)):
                        nc.tensor.matmul(
                            ps_y[:, q, :],
                            xT0[:, q, c * P : (c + 1) * P],
                            w_mm[:, c * OUT_DIM : (c + 1) * OUT_DIM],
                            start=(i == 0),
                            stop=(i == 1),
                        )
                return ps_y

            def issue_y(pv, ps_y):
                g0, t0, _, _ = pv
                y_out = y_tiles[g0]
                # distribute y = PSUM + bias over DVE and the idle GPSIMD
                y_eng = nc.gpsimd if t0 < y_gp else nc.vector
                y_eng.tensor_add(
                    y_out[:, t0 * nb : (t0 + 1) * nb, :], ps_y[:], bias_bc[:]
                )
                # flush finished slices of y_out as soon as they complete
                per = nt // out_split
                if (t0 + 1) % per == 0:
                    sl = slice((t0 + 1 - per) * nb, (t0 + 1) * nb)
                    eng(out_dma).dma_start(y_v[g0][:, sl, :], y_out[:, sl, :])
                if t0 == nt - 1:
                    del y_tiles[g0]

            def cast_op(engine, dst, src):
                if engine == "scalar":
                    nc.scalar.activation(
                        dst, src, mybir.ActivationFunctionType.Copy
                    )
                elif engine == "gpsimd":
                    nc.gpsimd.tensor_copy(dst, src)
                else:
                    nc.vector.tensor_copy(dst, src)

            flat = [(g % n_groups, t) for g in range(n_groups * repeats) for t in range(nt)]
            cast_per = nt // cast_split  # t-blocks per cast slice
            cursor = 0

            def advance_cast(upto):
                # issue in-DMA + cast slices ahead of the compute cursor so
                # the ACT stream interleaves [move, cast-slice] instead of
                # stalling moves behind a whole-group cast blob
                nonlocal cursor
                while cursor <= upto and cursor < len(flat):
                    gg, tt = flat[cursor]
                    if tt == 0:
                        x_in = in_pool.tile([P, G, IN_DIM], F32, name="x_in")
                        eng(in_dma).dma_start(x_in[:], x_v[gg])
                        xh_tiles[gg] = (
                            xh_pool.tile([P, G, IN_DIM], mm_dt, name="xh"),
                            x_in,
                        )
                        y_tiles[gg] = out_pool.tile(
                            [P, G, OUT_DIM], y_dt, name="y_out", tag="y_out"
                        )
                    if tt % cast_per == 0:
                        xh, x_in = xh_tiles[gg]
                        sl = slice(tt * nb, (tt + cast_per) * nb)
                        cast_op(cast_eng, xh[:, sl, :], x_in[:, sl, :])
                    cursor += 1

            for idx, (g, t) in enumerate(flat):
                advance_cast(idx + cast_lead)
                xh = xh_tiles[g][0]
                # PE: transposes of (g, t)
                ps_t = psum_t_pool.tile([P, nb, IN_DIM], mm_dt, tag="ps_t")
                for q in range(nb):
                    for c in range(2):
                        nc.tensor.transpose(
                            ps_t[:, q, c * P : (c + 1) * P],
                            xh[:, t * nb + q, c * P : (c + 1) * P],
                            ident[:],
                        )
                ps_y_prev = None
                prev = None
                if len(pending) >= pipe:
                    prev = pending.pop(0)
                    ps_y_prev = issue_matmuls(prev)
                # DVE/ACT: move xT of (g, t) out of PSUM (move_act of them
                # per group go to ACT, spread evenly across t-blocks)
                xT = xt_pool.tile([P, nb, IN_DIM], mm_dt)
                use_act = move_eng == "scalar" or (
                    move_act > 0 and t % (nt // move_act) == 0
                )
                if not use_act:
                    nc.vector.tensor_copy(xT[:], ps_t[:])
                else:
                    nc.scalar.activation(
                        xT[:], ps_t[:], mybir.ActivationFunctionType.Copy
                    )
                if prev is not None:
                    issue_y(prev, ps_y_prev)
                pending.append((g, t, xT, ps_t))
            for prev in pending:
                ps_y_prev = issue_matmuls(prev)
                issue_y(prev, ps_y_prev)
    nc.compile()
    return nc


_NC_CACHE: dict = {}

KERNEL_KWARGS: dict = {"y_fp16": True}


def _get_nc():
    if "nc" not in _NC_CACHE:
        _NC_CACHE["nc"] = build_bass_v2(**KERNEL_KWARGS)
    return _NC_CACHE["nc"]


def kernel(x, shift, sign, bias):
    x = np.ascontiguousarray(x, dtype=np.float32)
    shift = np.ascontiguousarray(shift, dtype=np.float32)
    sign = np.ascontiguousarray(sign, dtype=np.float32)
    bias = np.ascontiguousarray(bias, dtype=np.float32)
    assert x.shape == (BATCH, IN_DIM)

    nc = _get_nc()
    shards = np.split(x, N_CORES, axis=0)
    in_maps = [
        {"x": shards[c], "shift": shift, "sign": sign, "bias": bias}
        for c in range(N_CORES)
    ]
    res = run_bass_kernel_spmd(nc, in_maps, core_ids=list(range(N_CORES)))
    return np.concatenate(
        [np.asarray(r["y"], dtype=np.float32) for r in res.results], axis=0
    )

